# revision 5
# baseline (speedup 1.0000x reference)
"""Trainium2 Bass kernel for nn_DropLearner (gnn_message_passing).

aug_edge_weight = sigmoid((logit(eps) + MLP([head|tail|rel])) / T)

Strategy (8 NeuronCores, data-parallel over edges):
  - Edges sharded 62500/core, padded to 63488 = 31 groups x 2048.
  - all_embed stored fp16 (256B rows), gathered per-edge via indirect DMA
    (int32 row indices), 2048 rows per gather instruction.
  - Gathered edge-major tiles are transposed feature-major via DMA xbar
    transpose (fp16), 128x128 blocks.
  - MLP: h.T[192, 512] accumulated in PSUM from 3 matmuls per 96-half:
    W1h.T @ headT + W1t.T @ tailT + Rb.T @ onehot(type); relu-copied to
    SBUF fp16; weight = W2 @ h via matmuls into packed PSUM rows.
  - Per-edge weights staged to DRAM, re-read as [128, 496] for bulk
    gating (Ln/sigmoid on the scalar engine).
Precision: fp16 embeddings/W1/Rb/W2, fp32 accumulation -> ~2.5e-3 max
relative error vs the fp32 reference.
"""
import sys
sys.path.insert(0, "/opt/trn_rl_repo")

import contextlib
import numpy as np

import concourse.bacc as bacc
import concourse.bass as bass
import concourse.mybir as mybir
import concourse.tile as tile
from concourse.bass_utils import run_bass_kernel_spmd

# ---- problem constants (hardcoded per task contract) ----
N_NODES = 100000
D = 128           # node dim
N_REL = 32
E = 500000
H = 192           # 3 * mlp_dim
TEMP = 0.5
BIAS = 1e-4

NCORES = 8
EC = E // NCORES          # 62500 edges per core
GROUP = 2048              # edges per gather group
NG = 31                   # groups per core
EP = NG * GROUP           # 63488 padded edges per core
KIDX = GROUP // 128       # 16 idx columns per group
F = EP // 128             # 496 columns in the [128, F] final layout
NCHUNK = NG * 4           # 124 chunks of 512

F16 = mybir.dt.float16
F32 = mybir.dt.float32
I32 = mybir.dt.int32

_CACHE = {}


def _build_program():
    nc = bacc.Bacc("TRN2", target_bir_lowering=False, debug=False,
                   num_devices=NCORES)
    tab = nc.dram_tensor("tab", [N_NODES, D // 2], I32, kind="ExternalInput").ap()
    idxh = nc.dram_tensor("idxh", [128, F], I32, kind="ExternalInput").ap()
    idxt = nc.dram_tensor("idxt", [128, F], I32, kind="ExternalInput").ap()
    onehot = nc.dram_tensor("onehot", [NG, N_REL, GROUP], F16, kind="ExternalInput").ap()
    u_in = nc.dram_tensor("u", [EP], F32, kind="ExternalInput").ap()
    w1ht = nc.dram_tensor("w1ht", [D, H], F16, kind="ExternalInput").ap()
    w1tt = nc.dram_tensor("w1tt", [D, H], F16, kind="ExternalInput").ap()
    rbt = nc.dram_tensor("rbt", [N_REL, H], F16, kind="ExternalInput").ap()
    w2c = nc.dram_tensor("w2c", [96, 2], F16, kind="ExternalInput").ap()
    b2b = nc.dram_tensor("b2b", [128, 1], F32, kind="ExternalInput").ap()
    gate = nc.dram_tensor("gate", [EP], F32, kind="ExternalOutput").ap()

    RELU = mybir.ActivationFunctionType.Relu
    LN = mybir.ActivationFunctionType.Ln
    SIG = mybir.ActivationFunctionType.Sigmoid

    with tile.TileContext(nc) as tc, contextlib.ExitStack() as ctx:
        constp = ctx.enter_context(tc.tile_pool(name="const", bufs=1))
        gathp = ctx.enter_context(tc.tile_pool(name="gath", bufs=2))
        onep = ctx.enter_context(tc.tile_pool(name="onep", bufs=2))
        xtp = ctx.enter_context(tc.tile_pool(name="xt", bufs=3))
        hps = ctx.enter_context(tc.tile_pool(name="hps", bufs=2, space="PSUM"))
        wps = ctx.enter_context(tc.tile_pool(name="wps", bufs=2, space="PSUM"))
        hsbp = ctx.enter_context(tc.tile_pool(name="hsb", bufs=3))
        wsbp = ctx.enter_context(tc.tile_pool(name="wsb", bufs=2))
        finp = ctx.enter_context(tc.tile_pool(name="fin", bufs=1))
        dramp = ctx.enter_context(tc.tile_pool(name="wdram", bufs=1, space="DRAM"))

        # constants / inputs resident in SBUF
        idxh_sb = constp.tile([128, F], I32, tag="idxh")
        idxt_sb = constp.tile([128, F], I32, tag="idxt")
        nc.sync.dma_start(out=idxh_sb[:], in_=idxh[:])
        nc.sync.dma_start(out=idxt_sb[:], in_=idxt[:])
        w1ht_sb = constp.tile([D, H], F16, tag="w1ht")
        w1tt_sb = constp.tile([D, H], F16, tag="w1tt")
        rbt_sb = constp.tile([N_REL, H], F16, tag="rbt")
        w2c_sb = constp.tile([96, 2], F16, tag="w2c")
        b2b_sb = constp.tile([128, 1], F32, tag="b2b")
        nc.sync.dma_start(out=w1ht_sb[:], in_=w1ht[:])
        nc.sync.dma_start(out=w1tt_sb[:], in_=w1tt[:])
        nc.sync.dma_start(out=rbt_sb[:], in_=rbt[:])
        nc.sync.dma_start(out=w2c_sb[:], in_=w2c[:])
        nc.sync.dma_start(out=b2b_sb[:], in_=b2b[:])

        w_dram = dramp.tile([EP], F32)

        for g in range(NG):
            gh = gathp.tile([128, KIDX * (D // 2)], I32, tag="gh")
            gt = gathp.tile([128, KIDX * (D // 2)], I32, tag="gt")
            # HW indirect DMA consumes ONE index per output partition, so each
            # call gathers 128 rows (one 256B row per partition).
            for j in range(KIDX):
                nc.gpsimd.indirect_dma_start(
                    out=gh[:, j * 64:(j + 1) * 64], out_offset=None, in_=tab[:],
                    in_offset=bass.IndirectOffsetOnAxis(
                        ap=idxh_sb[:, g * KIDX + j:g * KIDX + j + 1], axis=0))
                nc.gpsimd.indirect_dma_start(
                    out=gt[:, j * 64:(j + 1) * 64], out_offset=None, in_=tab[:],
                    in_offset=bass.IndirectOffsetOnAxis(
                        ap=idxt_sb[:, g * KIDX + j:g * KIDX + j + 1], axis=0))
            oh = onep.tile([N_REL, GROUP], F16, tag="oh")
            nc.sync.dma_start(out=oh[:], in_=onehot[g])

            wp = wps.tile([128, 512], F32, tag="wp")
            nc.vector.memset(wp[:], 0.0)
            for s in range(4):
                xh = xtp.tile([128, 512], F16, tag="xh")
                xt_ = xtp.tile([128, 512], F16, tag="xt")
                for b in range(4):
                    blk = 4 * s + b
                    nc.sync.dma_start(
                        out=xh[:, b * 128:(b + 1) * 128],
                        in_=gh[:, blk * 64:(blk + 1) * 64].bitcast(F16),
                        transpose=True)
                    nc.sync.dma_start(
                        out=xt_[:, b * 128:(b + 1) * 128],
                        in_=gt[:, blk * 64:(blk + 1) * 64].bitcast(F16),
                        transpose=True)
                hsb = hsbp.tile([96, 1024], F16, tag="hsb")
                for half in range(2):
                    c0 = half * 96
                    hp = hps.tile([96, 512], F32, tag=f"h{half}")
                    nc.tensor.matmul(out=hp[:], lhsT=w1ht_sb[:, c0:c0 + 96],
                                     rhs=xh[:], start=True, stop=False)
                    nc.tensor.matmul(out=hp[:], lhsT=w1tt_sb[:, c0:c0 + 96],
                                     rhs=xt_[:], start=False, stop=False)
                    nc.tensor.matmul(out=hp[:], lhsT=rbt_sb[:, c0:c0 + 96],
                                     rhs=oh[:, s * 512:(s + 1) * 512],
                                     start=False, stop=True)
                    nc.scalar.activation(out=hsb[:, half * 512:(half + 1) * 512],
                                         in_=hp[:], func=RELU)
                nc.tensor.matmul(out=wp[32 * s:32 * s + 1, :],
                                 lhsT=w2c_sb[:, 0:1], rhs=hsb[:, :512],
                                 start=True, stop=False, tile_position=(0, 32 * s))
                nc.tensor.matmul(out=wp[32 * s:32 * s + 1, :],
                                 lhsT=w2c_sb[:, 1:2], rhs=hsb[:, 512:],
                                 start=False, stop=True, tile_position=(0, 32 * s))
            w_sb = wsbp.tile([128, 512], F32, tag="wsb")
            nc.vector.tensor_copy(out=w_sb[:], in_=wp[:])
            nc.sync.dma_start(
                out=w_dram[g * GROUP:(g + 1) * GROUP].rearrange("(a b) -> a b", a=4),
                in_=w_sb[0:128:32, :])

        tc.strict_bb_all_engine_barrier()

        # final gating: gate = sigmoid(2*(ln(eps) - ln(1-eps) + w + b2))
        wst = finp.tile([128, F], F32, tag="wst")
        ut = finp.tile([128, F], F32, tag="ut")
        l1 = finp.tile([128, F], F32, tag="l1")
        l2 = finp.tile([128, F], F32, tag="l2")
        gt_ = finp.tile([128, F], F32, tag="gt")
        lnb1 = finp.tile([128, 1], F32, tag="lnb1")
        lnb2 = finp.tile([128, 1], F32, tag="lnb2")
        nc.vector.memset(lnb1[:], float(1.0 - BIAS))
        nc.vector.memset(lnb2[:], float(BIAS))
        nc.sync.dma_start(out=wst[:], in_=w_dram[:].rearrange("(p f) -> p f", p=128))
        nc.sync.dma_start(out=ut[:], in_=u_in[:].rearrange("(p f) -> p f", p=128))
        nc.scalar.activation(out=l1[:], in_=ut[:], func=LN,
                             scale=float(2.0 * BIAS - 1.0), bias=lnb1[:])
        nc.scalar.activation(out=l2[:], in_=ut[:], func=LN,
                             scale=float(1.0 - 2.0 * BIAS), bias=lnb2[:])
        nc.vector.tensor_tensor(out=l1[:], in0=l1[:], in1=l2[:],
                                op=mybir.AluOpType.subtract)
        nc.vector.tensor_tensor(out=l1[:], in0=l1[:], in1=wst[:],
                                op=mybir.AluOpType.add)
        nc.scalar.activation(out=gt_[:], in_=l1[:], func=SIG,
                             scale=float(1.0 / TEMP), bias=b2b_sb[:])
        nc.sync.dma_start(out=gate[:].rearrange("(p f) -> p f", p=128), in_=gt_[:])

    nc.compile()
    return nc


def _pos_to_e():
    """Device output position -> padded edge index, per core."""
    pos = np.arange(EP)
    g, r = pos // GROUP, pos % GROUP
    s, r2 = r // 512, r % 512
    b, p = r2 // 128, r2 % 128
    return g * GROUP + 16 * p + 4 * s + b  # e(g,s,b,p) with j = 4s+b


def _prep(edge_index, edge_type, all_embed, relation_emb, u, W1, b1, W2, b2):
    tab16 = np.ascontiguousarray(all_embed.astype(np.float16)).view(np.int32)
    W1 = np.asarray(W1, np.float32)
    w1ht = np.ascontiguousarray(W1[:, :D].T.astype(np.float16))
    w1tt = np.ascontiguousarray(W1[:, D:2 * D].T.astype(np.float16))
    rb = np.asarray(relation_emb, np.float32) @ W1[:, 2 * D:].T + np.asarray(b1, np.float32)
    rbt = np.ascontiguousarray(rb.T.astype(np.float16).T)  # [32, 192] fp16
    W2 = np.asarray(W2, np.float32)
    w2c = np.ascontiguousarray(np.stack([W2[0, :96], W2[0, 96:]], axis=1).astype(np.float16))
    b2b = np.full((128, 1), 2.0 * float(np.asarray(b2).reshape(-1)[0]), np.float32)

    head = np.asarray(edge_index[0], np.int64).astype(np.int32)
    tail = np.asarray(edge_index[1], np.int64).astype(np.int32)
    etype = np.asarray(edge_type, np.int64).astype(np.int32)
    u = np.asarray(u, np.float32)
    e2p = _pos_to_e()

    in_maps = []
    for c in range(NCORES):
        sl = slice(c * EC, (c + 1) * EC)
        hp = np.zeros(EP, np.int32); hp[:EC] = head[sl]
        tp = np.zeros(EP, np.int32); tp[:EC] = tail[sl]
        ep_ = np.zeros(EP, np.int32); ep_[:EC] = etype[sl]
        up = np.full(EP, 0.5, np.float32); up[:EC] = u[sl]
        idxh = np.ascontiguousarray(hp.reshape(NG, 128, KIDX).transpose(1, 0, 2).reshape(128, F))
        idxt = np.ascontiguousarray(tp.reshape(NG, 128, KIDX).transpose(1, 0, 2).reshape(128, F))
        t_pos = ep_[e2p]
        onehot = (t_pos.reshape(NG, 1, GROUP) ==
                  np.arange(N_REL, dtype=np.int32).reshape(1, N_REL, 1)).astype(np.float16)
        u_dev = up[e2p]
        in_maps.append({
            "tab": tab16, "idxh": idxh, "idxt": idxt,
            "onehot": onehot, "u": u_dev,
            "w1ht": w1ht, "w1tt": w1tt, "rbt": rbt, "w2c": w2c, "b2b": b2b,
        })
    return in_maps, e2p


def kernel(edge_index, edge_type, all_embed, relation_emb, u, W1, b1, W2, b2):
    if "nc" not in _CACHE:
        _CACHE["nc"] = _build_program()
    nc = _CACHE["nc"]
    in_maps, e2p = _prep(edge_index, edge_type, all_embed, relation_emb, u,
                         W1, b1, W2, b2)
    res = run_bass_kernel_spmd(nc, in_maps, list(range(NCORES)))
    out = np.empty(E, np.float32)
    for c in range(NCORES):
        gate_pos = res.results[c]["gate"]          # [EP] in pos order
        core = np.empty(EP, np.float32)
        core[e2p] = gate_pos
        out[c * EC:(c + 1) * EC] = core[:EC]
    return out


# revision 8
# speedup vs baseline: 6.3523x; 6.3523x over previous
"""Trainium2 Bass kernel for nn_DropLearner (gnn_message_passing).

aug_edge_weight = sigmoid((logit(eps) + MLP([head|tail|rel])) / T)

Strategy (8 NeuronCores, data-parallel over edges):
  - Edges sharded 62500/core, padded to 63488 = 31 groups x 2048.
  - all_embed stored fp16 (256B rows), gathered per-edge via indirect DMA
    (int32 row indices), 2048 rows per gather instruction.
  - Gathered edge-major tiles are transposed feature-major via DMA xbar
    transpose (fp16), 128x128 blocks.
  - MLP: h.T[192, 512] accumulated in PSUM from 3 matmuls per 96-half:
    W1h.T @ headT + W1t.T @ tailT + Rb.T @ onehot(type); relu-copied to
    SBUF fp16; weight = W2 @ h via matmuls into packed PSUM rows.
  - Per-edge weights staged to DRAM, re-read as [128, 496] for bulk
    gating (Ln/sigmoid on the scalar engine).
Precision: fp16 embeddings/W1/Rb/W2, fp32 accumulation -> ~2.5e-3 max
relative error vs the fp32 reference.
"""
import sys
sys.path.insert(0, "/opt/trn_rl_repo")

import contextlib
import numpy as np

import concourse.bacc as bacc
import concourse.bass as bass
import concourse.mybir as mybir
import concourse.tile as tile
from concourse.bass_utils import run_bass_kernel_spmd

# ---- problem constants (hardcoded per task contract) ----
N_NODES = 100000
D = 128           # node dim
N_REL = 32
E = 500000
H = 192           # 3 * mlp_dim
TEMP = 0.5
BIAS = 1e-4

NCORES = 8
EC = E // NCORES          # 62500 edges per core
GROUP = 2048              # edges per gather group
NG = 31                   # groups per core
EP = NG * GROUP           # 63488 padded edges per core
KIDX = GROUP // 128       # 16 idx columns per group
F = EP // 128             # 496 columns in the [128, F] final layout
NCHUNK = NG * 4           # 124 chunks of 512

F16 = mybir.dt.float16
F32 = mybir.dt.float32
I32 = mybir.dt.int32

_CACHE = {}


def _build_program():
    nc = bacc.Bacc("TRN2", target_bir_lowering=False, debug=False,
                   num_devices=NCORES)
    tab = nc.dram_tensor("tab", [N_NODES, D // 2], I32, kind="ExternalInput").ap()
    idxh = nc.dram_tensor("idxh", [128, F], I32, kind="ExternalInput").ap()
    idxt = nc.dram_tensor("idxt", [128, F], I32, kind="ExternalInput").ap()
    onehot = nc.dram_tensor("onehot", [NG, N_REL, GROUP], F16, kind="ExternalInput").ap()
    u_in = nc.dram_tensor("u", [EP], F32, kind="ExternalInput").ap()
    w1ht = nc.dram_tensor("w1ht", [D, H], F16, kind="ExternalInput").ap()
    w1tt = nc.dram_tensor("w1tt", [D, H], F16, kind="ExternalInput").ap()
    rbt = nc.dram_tensor("rbt", [N_REL, H], F16, kind="ExternalInput").ap()
    w2c = nc.dram_tensor("w2c", [96, 2], F16, kind="ExternalInput").ap()
    b2b = nc.dram_tensor("b2b", [128, 1], F32, kind="ExternalInput").ap()
    gate = nc.dram_tensor("gate", [EP], F32, kind="ExternalOutput").ap()

    RELU = mybir.ActivationFunctionType.Relu
    LN = mybir.ActivationFunctionType.Ln
    SIG = mybir.ActivationFunctionType.Sigmoid

    with tile.TileContext(nc) as tc, contextlib.ExitStack() as ctx:
        constp = ctx.enter_context(tc.tile_pool(name="const", bufs=1))
        gathp = ctx.enter_context(tc.tile_pool(name="gath", bufs=2))
        onep = ctx.enter_context(tc.tile_pool(name="onep", bufs=2))
        xtp = ctx.enter_context(tc.tile_pool(name="xt", bufs=3))
        hps = ctx.enter_context(tc.tile_pool(name="hps", bufs=2, space="PSUM"))
        wps = ctx.enter_context(tc.tile_pool(name="wps", bufs=2, space="PSUM"))
        xpp = ctx.enter_context(tc.tile_pool(name="xpp", bufs=2, space="PSUM"))
        hsbp = ctx.enter_context(tc.tile_pool(name="hsb", bufs=3))
        wsbp = ctx.enter_context(tc.tile_pool(name="wsb", bufs=2))
        finp = ctx.enter_context(tc.tile_pool(name="fin", bufs=1))
        dramp = ctx.enter_context(tc.tile_pool(name="wdram", bufs=1, space="DRAM"))

        # constants / inputs resident in SBUF
        idxh_sb = constp.tile([128, F], I32, tag="idxh")
        idxt_sb = constp.tile([128, F], I32, tag="idxt")
        nc.sync.dma_start(out=idxh_sb[:], in_=idxh[:])
        nc.sync.dma_start(out=idxt_sb[:], in_=idxt[:])
        w1ht_sb = constp.tile([D, H], F16, tag="w1ht")
        w1tt_sb = constp.tile([D, H], F16, tag="w1tt")
        rbt_sb = constp.tile([N_REL, H], F16, tag="rbt")
        w2c_sb = constp.tile([96, 2], F16, tag="w2c")
        b2b_sb = constp.tile([128, 1], F32, tag="b2b")
        ident = constp.tile([128, 128], F16, tag="ident")
        from concourse.masks import make_identity
        make_identity(nc, ident[:])
        nc.sync.dma_start(out=w1ht_sb[:], in_=w1ht[:])
        nc.sync.dma_start(out=w1tt_sb[:], in_=w1tt[:])
        nc.sync.dma_start(out=rbt_sb[:], in_=rbt[:])
        nc.sync.dma_start(out=w2c_sb[:], in_=w2c[:])
        nc.sync.dma_start(out=b2b_sb[:], in_=b2b[:])

        w_dram = dramp.tile([EP], F32)

        for g in range(NG):
            gh = gathp.tile([128, KIDX * (D // 2)], I32, tag="gh")
            gt = gathp.tile([128, KIDX * (D // 2)], I32, tag="gt")
            # HW indirect DMA consumes ONE index per output partition, so each
            # call gathers 128 rows (one 256B row per partition).
            for j in range(KIDX):
                nc.gpsimd.indirect_dma_start(
                    out=gh[:, j * 64:(j + 1) * 64], out_offset=None, in_=tab[:],
                    in_offset=bass.IndirectOffsetOnAxis(
                        ap=idxh_sb[:, g * KIDX + j:g * KIDX + j + 1], axis=0))
                nc.gpsimd.indirect_dma_start(
                    out=gt[:, j * 64:(j + 1) * 64], out_offset=None, in_=tab[:],
                    in_offset=bass.IndirectOffsetOnAxis(
                        ap=idxt_sb[:, g * KIDX + j:g * KIDX + j + 1], axis=0))
            oh = onep.tile([N_REL, GROUP], F16, tag="oh")
            nc.sync.dma_start(out=oh[:], in_=onehot[g])

            wp = wps.tile([128, 512], F32, tag="wp")
            nc.vector.memset(wp[:], 0.0)
            for s in range(4):
                # PE transposes: 8 x [128,128] fp16 into one PSUM bank
                xps = xpp.tile([128, 1024], F16, tag="xps")
                for b in range(4):
                    blk = 4 * s + b
                    nc.tensor.transpose(
                        out=xps[:, b * 128:(b + 1) * 128],
                        in_=gh[:, blk * 64:(blk + 1) * 64].bitcast(F16),
                        identity=ident[:])
                    nc.tensor.transpose(
                        out=xps[:, 512 + b * 128:512 + (b + 1) * 128],
                        in_=gt[:, blk * 64:(blk + 1) * 64].bitcast(F16),
                        identity=ident[:])
                xsb = xtp.tile([128, 1024], F16, tag="xsb")
                nc.vector.tensor_copy(out=xsb[:], in_=xps[:])
                xh = xsb[:, :512]
                xt_ = xsb[:, 512:]
                hsb = hsbp.tile([96, 1024], F16, tag="hsb")
                for half in range(2):
                    c0 = half * 96
                    hp = hps.tile([96, 512], F32, tag=f"h{half}")
                    nc.tensor.matmul(out=hp[:], lhsT=w1ht_sb[:, c0:c0 + 96],
                                     rhs=xh[:], start=True, stop=False)
                    nc.tensor.matmul(out=hp[:], lhsT=w1tt_sb[:, c0:c0 + 96],
                                     rhs=xt_[:], start=False, stop=False)
                    nc.tensor.matmul(out=hp[:], lhsT=rbt_sb[:, c0:c0 + 96],
                                     rhs=oh[:, s * 512:(s + 1) * 512],
                                     start=False, stop=True)
                    nc.scalar.activation(out=hsb[:, half * 512:(half + 1) * 512],
                                         in_=hp[:], func=RELU)
                nc.tensor.matmul(out=wp[32 * s:32 * s + 1, :],
                                 lhsT=w2c_sb[:, 0:1], rhs=hsb[:, :512],
                                 start=True, stop=False, tile_position=(0, 32 * s))
                nc.tensor.matmul(out=wp[32 * s:32 * s + 1, :],
                                 lhsT=w2c_sb[:, 1:2], rhs=hsb[:, 512:],
                                 start=False, stop=True, tile_position=(0, 32 * s))
            w_sb = wsbp.tile([128, 512], F32, tag="wsb")
            nc.vector.tensor_copy(out=w_sb[:], in_=wp[:])
            nc.sync.dma_start(
                out=w_dram[g * GROUP:(g + 1) * GROUP].rearrange("(a b) -> a b", a=4),
                in_=w_sb[0:128:32, :])

        tc.strict_bb_all_engine_barrier()

        # final gating: gate = sigmoid(2*(ln(eps) - ln(1-eps) + w + b2))
        wst = finp.tile([128, F], F32, tag="wst")
        ut = finp.tile([128, F], F32, tag="ut")
        l1 = finp.tile([128, F], F32, tag="l1")
        l2 = finp.tile([128, F], F32, tag="l2")
        gt_ = finp.tile([128, F], F32, tag="gt")
        lnb1 = finp.tile([128, 1], F32, tag="lnb1")
        lnb2 = finp.tile([128, 1], F32, tag="lnb2")
        nc.vector.memset(lnb1[:], float(1.0 - BIAS))
        nc.vector.memset(lnb2[:], float(BIAS))
        nc.sync.dma_start(out=wst[:], in_=w_dram[:].rearrange("(p f) -> p f", p=128))
        nc.sync.dma_start(out=ut[:], in_=u_in[:].rearrange("(p f) -> p f", p=128))
        nc.scalar.activation(out=l1[:], in_=ut[:], func=LN,
                             scale=float(2.0 * BIAS - 1.0), bias=lnb1[:])
        nc.scalar.activation(out=l2[:], in_=ut[:], func=LN,
                             scale=float(1.0 - 2.0 * BIAS), bias=lnb2[:])
        nc.vector.tensor_tensor(out=l1[:], in0=l1[:], in1=l2[:],
                                op=mybir.AluOpType.subtract)
        nc.vector.tensor_tensor(out=l1[:], in0=l1[:], in1=wst[:],
                                op=mybir.AluOpType.add)
        nc.scalar.activation(out=gt_[:], in_=l1[:], func=SIG,
                             scale=float(1.0 / TEMP), bias=b2b_sb[:])
        nc.sync.dma_start(out=gate[:].rearrange("(p f) -> p f", p=128), in_=gt_[:])

    nc.compile()
    return nc


def _pos_to_e():
    """Device output position -> padded edge index, per core."""
    pos = np.arange(EP)
    g, r = pos // GROUP, pos % GROUP
    s, r2 = r // 512, r % 512
    b, p = r2 // 128, r2 % 128
    return g * GROUP + 16 * p + 4 * s + b  # e(g,s,b,p) with j = 4s+b


def _prep(edge_index, edge_type, all_embed, relation_emb, u, W1, b1, W2, b2):
    tab16 = np.ascontiguousarray(all_embed.astype(np.float16)).view(np.int32)
    W1 = np.asarray(W1, np.float32)
    w1ht = np.ascontiguousarray(W1[:, :D].T.astype(np.float16))
    w1tt = np.ascontiguousarray(W1[:, D:2 * D].T.astype(np.float16))
    rb = np.asarray(relation_emb, np.float32) @ W1[:, 2 * D:].T + np.asarray(b1, np.float32)
    rbt = np.ascontiguousarray(rb.T.astype(np.float16).T)  # [32, 192] fp16
    W2 = np.asarray(W2, np.float32)
    w2c = np.ascontiguousarray(np.stack([W2[0, :96], W2[0, 96:]], axis=1).astype(np.float16))
    b2b = np.full((128, 1), 2.0 * float(np.asarray(b2).reshape(-1)[0]), np.float32)

    head = np.asarray(edge_index[0], np.int64).astype(np.int32)
    tail = np.asarray(edge_index[1], np.int64).astype(np.int32)
    etype = np.asarray(edge_type, np.int64).astype(np.int32)
    u = np.asarray(u, np.float32)
    e2p = _pos_to_e()

    in_maps = []
    for c in range(NCORES):
        sl = slice(c * EC, (c + 1) * EC)
        hp = np.zeros(EP, np.int32); hp[:EC] = head[sl]
        tp = np.zeros(EP, np.int32); tp[:EC] = tail[sl]
        ep_ = np.zeros(EP, np.int32); ep_[:EC] = etype[sl]
        up = np.full(EP, 0.5, np.float32); up[:EC] = u[sl]
        idxh = np.ascontiguousarray(hp.reshape(NG, 128, KIDX).transpose(1, 0, 2).reshape(128, F))
        idxt = np.ascontiguousarray(tp.reshape(NG, 128, KIDX).transpose(1, 0, 2).reshape(128, F))
        t_pos = ep_[e2p]
        onehot = (t_pos.reshape(NG, 1, GROUP) ==
                  np.arange(N_REL, dtype=np.int32).reshape(1, N_REL, 1)).astype(np.float16)
        u_dev = up[e2p]
        in_maps.append({
            "tab": tab16, "idxh": idxh, "idxt": idxt,
            "onehot": onehot, "u": u_dev,
            "w1ht": w1ht, "w1tt": w1tt, "rbt": rbt, "w2c": w2c, "b2b": b2b,
        })
    return in_maps, e2p


def kernel(edge_index, edge_type, all_embed, relation_emb, u, W1, b1, W2, b2):
    if "nc" not in _CACHE:
        _CACHE["nc"] = _build_program()
    nc = _CACHE["nc"]
    in_maps, e2p = _prep(edge_index, edge_type, all_embed, relation_emb, u,
                         W1, b1, W2, b2)
    res = run_bass_kernel_spmd(nc, in_maps, list(range(NCORES)))
    out = np.empty(E, np.float32)
    for c in range(NCORES):
        gate_pos = res.results[c]["gate"]          # [EP] in pos order
        core = np.empty(EP, np.float32)
        core[e2p] = gate_pos
        out[c * EC:(c + 1) * EC] = core[:EC]
    return out
